# revision 1
# baseline (speedup 1.0000x reference)
"""DeltaNet (chunked delta rule) Trainium2 kernel.

Sharding: B*H = 32 (batch, head) recurrence states -> 8 cores, each core
owns one batch and 4 heads (data + head-tensor parallel). Projections for
beta/gate are computed on-device per core from that batch's hidden states.

Device math per (chunk n, head h), chunk size C=128 (the delta-rule chunked
algorithm is chunk-size invariant; reference uses 64):
  G'    = k k^T                       (PE, bf16 operands, f32 accum)
  X     = -strict_lower(diag(beta) G')
  TmT   = ((I + X)(I + X^2)...(I + X^32))^T  via Y = X^T power chain
          (X^64 term dropped: |X| < 1 so X^64 ~ 1e-8, far below bf16 noise)
  attnT = triu(k q^T)  (incl diag)
  wTn   = (-k_beta)^T TmT = -(Tm k_beta)^T
  vi    = Tm v_beta - (Tm k_beta) S    (one PSUM accumulation)
  o     = q S + attn vi                (one PSUM accumulation)
  S    += k^T vi                       (f32 master in SBUF, delta via PSUM)
  out   = RMSNorm(o) * silu(g) ; outT = W_o^T o^T  (per-head projection)

Each head gets its own SBUF/PSUM tile tags so the 4 head pipelines run
concurrently across engines (PSUM: 4 head tags x 2 bufs = 8 banks).
"""

import os
import sys

sys.path.insert(0, "/opt/trn_rl_repo")

import numpy as np
import ml_dtypes
from contextlib import ExitStack

B, T, H, DK, DV, HID = 2, 4096, 16, 128, 128, 2048
C = 128
NCH = T // C          # 32 chunks
HL = 4                # heads per core
NCORES = 8
KT = HID // 128       # 16 hidden k-tiles
EPS = 1e-5
BF = ml_dtypes.bfloat16

_CACHE = {}


def _build_nc(nch, run_nch=None):
    import concourse.bass as bass
    from concourse import bacc
    import concourse.tile as tile
    from concourse import mybir

    f32 = mybir.dt.float32
    bf16 = mybir.dt.bfloat16
    AF = mybir.ActivationFunctionType
    MUL = mybir.AluOpType.mult
    ADD = mybir.AluOpType.add
    t = nch * C
    if run_nch is None:
        run_nch = nch

    nc = bacc.Bacc()
    # qkv packs (kT, qT, kN, vN) [128,128] blocks per (head, chunk)
    qkv = nc.dram_tensor("qkv", (HL, nch, 4, 128, 128), bf16, kind="ExternalInput")
    habt = nc.dram_tensor("habt", (KT, 128, t), bf16, kind="ExternalInput")
    hgt = nc.dram_tensor("hgt", (KT, 128, t), bf16, kind="ExternalInput")
    wb = nc.dram_tensor("wb", (KT, 128, HL), bf16, kind="ExternalInput")
    wg = nc.dram_tensor("wg", (KT, 128, HL), bf16, kind="ExternalInput")
    wo = nc.dram_tensor("wo", (HL, DV, DK), bf16, kind="ExternalInput")
    ident = nc.dram_tensor("ident", (128, 128), bf16, kind="ExternalInput")
    mlow = nc.dram_tensor("mlow", (128, 128), f32, kind="ExternalInput")
    mtriu = nc.dram_tensor("mtriu", (128, 128), f32, kind="ExternalInput")
    outt = nc.dram_tensor("outt", (HL, DK, t), f32, kind="ExternalOutput")

    with tile.TileContext(nc) as tc, ExitStack() as ctx:
        consts = ctx.enter_context(tc.tile_pool(name="consts", bufs=1))
        hidp = ctx.enter_context(tc.tile_pool(name="hid", bufs=4))
        main = ctx.enter_context(tc.tile_pool(name="main", bufs=2))
        smallp = ctx.enter_context(tc.tile_pool(name="small", bufs=4))
        persist = ctx.enter_context(tc.tile_pool(name="persist", bufs=1))
        dram = ctx.enter_context(tc.tile_pool(name="dram", bufs=1, space="DRAM"))
        pwork = ctx.enter_context(tc.tile_pool(name="pwork", bufs=2, space="PSUM"))

        # ---- constants ----
        ident_s = consts.tile([128, 128], bf16)
        nc.sync.dma_start(ident_s, ident[:])
        mlow_s = consts.tile([128, 128], f32)
        nc.sync.dma_start(mlow_s, mlow[:])
        mtriu_s = consts.tile([128, 128], f32)
        nc.sync.dma_start(mtriu_s, mtriu[:])
        wb_s = consts.tile([128, KT, HL], bf16)
        nc.sync.dma_start(wb_s, wb.rearrange("k p h -> p k h"))
        wg_s = consts.tile([128, KT, HL], bf16)
        nc.sync.dma_start(wg_s, wg.rearrange("k p h -> p k h"))
        wo_s = consts.tile([128, HL, DK], bf16)
        nc.sync.dma_start(wo_s, wo.rearrange("h v d -> v h d"))
        eps_t = consts.tile([128, 1], f32)
        nc.vector.memset(eps_t, EPS)

        # ---- phase 1: beta/g projection logits -> DRAM scratch ----
        beta_scr = dram.tile([HL, t], f32)
        g_scr = dram.tile([HL, t], f32)
        ntt = t // 512
        pi = 0
        for scr, hidt, w_s in ((beta_scr, habt, wb_s), (g_scr, hgt, wg_s)):
            for tt in range(ntt):
                ps = pwork.tile([4, 512], f32, tag=f"w{pi % 4}", name="ps")
                pi += 1
                for k in range(KT):
                    hb = hidp.tile([128, 512], bf16, tag="hid")
                    dmae = nc.sync if k % 2 else nc.gpsimd
                    dmae.dma_start(hb, hidt[k, :, tt * 512:(tt + 1) * 512])
                    nc.tensor.matmul(ps, w_s[:, k, :], hb,
                                     start=(k == 0), stop=(k == KT - 1))
                sb = smallp.tile([4, 512], f32, tag="blog")
                nc.scalar.copy(sb, ps)
                nc.sync.dma_start(scr[:, tt * 512:(tt + 1) * 512], sb)

        # ---- phase 1b: reload per head in [128, nch] layout; gates ----
        bpos, bneg, gsil = [], [], []
        for h in range(HL):
            bl = smallp.tile([128, nch], f32, tag="bload")
            nc.gpsimd.dma_start(bl, beta_scr[h].rearrange("(n p) -> p n", p=128))
            bp = persist.tile([128, nch], f32, tag=f"bp{h}")
            nc.scalar.activation(bp, bl, AF.Sigmoid)
            bn = persist.tile([128, nch], f32, tag=f"bn{h}")
            nc.vector.tensor_scalar_mul(bn, bp, -1.0)
            gl = smallp.tile([128, nch], f32, tag="gload")
            nc.gpsimd.dma_start(gl, g_scr[h].rearrange("(n p) -> p n", p=128))
            gsg = smallp.tile([128, nch], f32, tag="gsg")
            nc.scalar.activation(gsg, gl, AF.Sigmoid)
            gs = persist.tile([128, nch], f32, tag=f"gs{h}")
            nc.vector.tensor_tensor(gs, gsg, gl, MUL)
            bpos.append(bp); bneg.append(bn); gsil.append(gs)

        # ---- persistent state ----
        S_sb = [persist.tile([128, DV], bf16, tag=f"Ssb{h}", name=f"Ssb{h}")
                for h in range(HL)]
        S_f32 = [None] * HL
        strip = [persist.tile([128, 4 * C], bf16, tag=f"strip{h}", name=f"strip{h}")
                 for h in range(HL)]

        # ---- phase 2: chunked scan, 4 independent head pipelines ----
        for n in range(run_nch):
            for h in range(HL):
                w = f"w{h}"
                qk = main.tile([128, 4, 128], bf16, tag=f"qk{h}", name="qk")
                dmae = nc.sync if (n + h) % 2 else nc.gpsimd
                dmae.dma_start(qk, qkv[h, n].rearrange("f p c -> p f c"))
                kT_ = qk[:, 0, :]
                qT_ = qk[:, 1, :]
                kN = qk[:, 2, :]
                vN = qk[:, 3, :]

                bn_ = bpos[h][:, n:n + 1]
                nb_ = bneg[h][:, n:n + 1]
                gt_ = gsil[h][:, n:n + 1]

                kbn = main.tile([C, DK], bf16, tag=f"kbn{h}", name="kbn")
                nc.gpsimd.tensor_scalar_mul(kbn, kN, nb_)
                vb = main.tile([C, DV], bf16, tag=f"vb{h}", name="vb")
                nc.gpsimd.tensor_scalar_mul(vb, vN, bn_)

                gp = pwork.tile([128, 128], f32, tag=w, name="gp")
                nc.tensor.matmul(gp, kT_, kT_, start=True, stop=True)
                xf = main.tile([128, 128], f32, tag=f"xf{h}", name="xf")
                nc.vector.tensor_scalar_mul(xf, gp, nb_)
                X1 = main.tile([128, 128], bf16, tag=f"X1{h}", name="X1")
                nc.gpsimd.tensor_tensor(X1, xf, mlow_s, MUL)
                pt = pwork.tile([128, 128], bf16, tag=w, name="pt")
                nc.tensor.transpose(pt, X1, ident_s)
                Y1 = main.tile([128, 128], bf16, tag=f"Y1{h}", name="Y1")
                nc.scalar.copy(Y1, pt)

                X = {1: X1}
                Y = {1: Y1}
                cp = 0
                for j in (2, 4, 8, 16, 32):
                    pj = pwork.tile([128, 128], f32, tag=w, name="pj")
                    nc.tensor.matmul(pj, Y[j // 2], X[j // 2], start=True, stop=True)
                    X[j] = main.tile([128, 128], bf16, tag=f"X{j}{h}", name=f"X{j}")
                    if cp % 2:
                        nc.scalar.copy(X[j], pj)
                    else:
                        nc.vector.tensor_copy(X[j], pj)
                    cp += 1
                    if j <= 16:
                        qj = pwork.tile([128, 128], f32, tag=w, name="qj")
                        nc.tensor.matmul(qj, X[j // 2], Y[j // 2], start=True, stop=True)
                        Y[j] = main.tile([128, 128], bf16, tag=f"Y{j}{h}", name=f"Y{j}")
                        if cp % 2:
                            nc.scalar.copy(Y[j], qj)
                        else:
                            nc.vector.tensor_copy(Y[j], qj)
                        cp += 1

                Tc = main.tile([128, 128], bf16, tag=f"T0{h}", name="T0")
                nc.gpsimd.tensor_tensor(Tc, Y1, ident_s, ADD)
                for i, j in enumerate((2, 4, 8, 16, 32)):
                    pp = pwork.tile([128, 128], f32, tag=w, name="pp")
                    nc.tensor.matmul(pp, X[j], Tc, start=True, stop=True)
                    Tn = main.tile([128, 128], bf16, tag=f"T{j}{h}", name=f"T{j}")
                    nc.vector.tensor_tensor(Tn, pp, Tc, ADD)
                    Tc = Tn
                TmT = Tc

                pa = pwork.tile([128, 128], f32, tag=w, name="pa")
                nc.tensor.matmul(pa, kT_, qT_, start=True, stop=True)
                attnT = main.tile([128, 128], bf16, tag=f"attnT{h}", name="attnT")
                nc.vector.tensor_tensor(attnT, pa, mtriu_s, MUL)

                pw_ = pwork.tile([128, 128], f32, tag=w, name="pw_")
                nc.tensor.matmul(pw_, kbn, TmT, start=True, stop=True)
                wTn = main.tile([128, 128], bf16, tag=f"wTn{h}", name="wTn")
                nc.scalar.copy(wTn, pw_)

                pvi = pwork.tile([128, 128], f32, tag=w, name="pvi")
                nc.tensor.matmul(pvi, TmT, vb, start=True, stop=(n == 0))
                if n > 0:
                    nc.tensor.matmul(pvi, wTn, S_sb[h], start=False, stop=True)
                vi = main.tile([128, 128], bf16, tag=f"vi{h}", name="vi")
                nc.vector.tensor_copy(vi, pvi)

                po = pwork.tile([128, 128], f32, tag=w, name="po")
                if n > 0:
                    nc.tensor.matmul(po, qT_, S_sb[h], start=True, stop=False)
                    nc.tensor.matmul(po, attnT, vi, start=False, stop=True)
                else:
                    nc.tensor.matmul(po, attnT, vi, start=True, stop=True)

                if n < nch - 1:
                    pds = pwork.tile([128, DV], f32, tag=w, name="pds")
                    nc.tensor.matmul(pds, kN, vi, start=True, stop=True)
                    Sf = main.tile([128, DV], f32, tag=f"Sf{h}", name=f"Sf{h}")
                    if n == 0:
                        nc.vector.tensor_copy(Sf, pds)
                    else:
                        nc.vector.tensor_tensor(Sf, pds, S_f32[h], ADD)
                    S_f32[h] = Sf
                    nc.gpsimd.tensor_copy(S_sb[h], Sf)

                # RMSNorm + gate (square+row-sum fused on scalar engine)
                o2d = main.tile([128, 128], bf16, tag=f"o2d{h}", name="o2d")
                sm = smallp.tile([128, 1], f32, tag=f"sm{h}", name="sm")
                nc.scalar.activation(o2d, po, AF.Square, accum_out=sm)
                sq = smallp.tile([128, 1], f32, tag=f"sq{h}", name="sq")
                nc.scalar.activation(sq, sm, AF.Sqrt, bias=eps_t, scale=1.0 / DV)
                rs = smallp.tile([128, 1], f32, tag=f"rs{h}", name="rs")
                nc.vector.reciprocal(rs, sq)
                onr = main.tile([128, 128], bf16, tag=f"onr{h}", name="onr")
                nc.vector.tensor_scalar(onr, po, rs, gt_, MUL, MUL)
                pot = pwork.tile([128, 128], bf16, tag=w, name="pot")
                nc.tensor.transpose(pot, onr, ident_s)
                nc.vector.tensor_copy(strip[h][:, (n % 4) * C:(n % 4 + 1) * C], pot)

                if n % 4 == 3:
                    pout = pwork.tile([128, 512], f32, tag=w, name="pout")
                    nc.tensor.matmul(pout, wo_s[:, h, :], strip[h],
                                     start=True, stop=True)
                    ofin = main.tile([128, 512], f32, tag=f"ofin{h}", name="ofin")
                    nc.vector.tensor_copy(ofin, pout)
                    nc.gpsimd.dma_start(outt[h][:, (n - 3) * C:(n + 1) * C], ofin)

    nc.compile()
    return nc


def _host_prep(hidden_ab, hidden_g, q, k, v, Wb, Wg, o_norm_w, o_proj_w, nch=NCH):
    """Shard + lay out inputs for the 8 cores. Returns list of in_maps."""
    t = nch * C

    def l2n(x):
        return x * (1.0 / np.sqrt(np.sum(x * x, -1, keepdims=True) + 1e-6))

    qn = l2n(q[:, :t].astype(np.float32)) * (DK ** -0.5)
    knrm = l2n(k[:, :t].astype(np.float32))
    vv = v[:, :t]

    ident = np.eye(128, dtype=BF)
    mlow = np.tril(np.ones((128, 128), np.float32), -1)
    mtriu = np.triu(np.ones((128, 128), np.float32), 0)

    in_maps = []
    for c in range(NCORES):
        b = c // 4
        h0 = (c % 4) * HL
        hs = slice(h0, h0 + HL)

        def chunks(x):
            return np.ascontiguousarray(
                x[b, :, hs].transpose(1, 0, 2).reshape(HL, nch, C, -1))

        qc = chunks(qn)
        kc = chunks(knrm)
        vc = chunks(vv)
        # pack (kT, qT, kN, vN) along a new axis -> [HL, nch, 4, 128, 128]
        qkv = np.stack([
            kc.transpose(0, 1, 3, 2), qc.transpose(0, 1, 3, 2), kc, vc,
        ], axis=2).astype(BF)
        habt = np.ascontiguousarray(hidden_ab[b, :t].T.reshape(KT, 128, t)).astype(BF)
        hgt = np.ascontiguousarray(hidden_g[b, :t].T.reshape(KT, 128, t)).astype(BF)
        in_maps.append(dict(
            qkv=qkv, habt=habt, hgt=hgt,
            wb=np.ascontiguousarray(Wb[:, hs].reshape(KT, 128, HL)).astype(BF),
            wg=np.ascontiguousarray(Wg[:, hs].reshape(KT, 128, HL)).astype(BF),
            wo=np.ascontiguousarray(o_proj_w[hs]).astype(BF),
            ident=ident, mlow=mlow, mtriu=mtriu,
        ))
    return in_maps


def _assemble(results, nch=NCH):
    t = nch * C
    out = np.zeros((B, t, H * DK), np.float32)
    for c, res in enumerate(results):
        b = c // 4
        h0 = (c % 4) * HL
        ot = res["outt"]  # [HL, DK, t]
        for hh in range(HL):
            out[b, :, (h0 + hh) * DK:(h0 + hh + 1) * DK] = ot[hh].T
    return out


def kernel(hidden_ab, hidden_g, q, k, v, Wb, Wg, o_norm_w, o_proj_w):
    from concourse.bass_utils import run_bass_kernel_spmd

    if "nc" not in _CACHE:
        _CACHE["nc"] = _build_nc(NCH)
    nc = _CACHE["nc"]
    in_maps = _host_prep(hidden_ab, hidden_g, q, k, v, Wb, Wg, o_norm_w, o_proj_w)
    res = run_bass_kernel_spmd(nc, in_maps, core_ids=list(range(NCORES)),
                               trace=bool(int(os.environ.get("DN_TRACE", "0"))))
    _CACHE["last_result"] = res
    return _assemble(res.results)

